# revision 26
# baseline (speedup 1.0000x reference)
"""Fused multi-head attention block (QKV -> softmax attention -> proj) on 8
TRN2 NeuronCores.

Sharding: data-parallel over batch (2) x tensor-parallel over heads (12 heads
-> 4 groups of 3). Core c handles batch c//4, heads 3*(c%4)..3*(c%4)+2.
Each core computes a rank-192 partial of the output projection; the host sums
the 4 partials per batch and adds proj bias.

v3 optimizations over the 552us baseline:
  - exp (the ScalarE bottleneck, ~408us) is split between ScalarE (true exp
    via ACT) and VectorE (Schraudolph bit-trick exp: one tensor_scalar
    computing round(a*s+b) into int16, bit-identical to an fp16 exp approx).
    Per-head softmax normalization absorbs the approximation's global bias.
  - PV matmuls split the K=128 j-contraction into two K=64 halves so (h0,h1)
    run as row-tiled concurrent pairs into one [65,1024] psum tile (same
    pattern the baseline proved for scores); h2 splits into separate column
    halves summed at drain. v keeps a ones column so PV also yields den.
  - score/PV matmuls grouped in bursts so LDWEIGHTS pipelines.
  - reciprocal via reciprocal_approx_fast batched over 2 i-blocks;
    normalization deferred (o drained unnormalized, scaled before proj).
"""

import numpy as np

import concourse.bass as bass
import concourse.mybir as mybir
import concourse.tile as tile
from concourse import bacc
from concourse.bass_utils import run_bass_kernel_spmd

I16 = mybir.dt.int16
F16 = mybir.dt.float16
F32 = mybir.dt.float32
EXP = mybir.ActivationFunctionType.Exp
MUL = mybir.AluOpType.mult
ADD = mybir.AluOpType.add

B = 2            # batch
N = 4096         # tokens (64*64)
C = 768          # channels
NH = 12          # heads
HD = 64          # head dim
HPC = 3          # heads per core
NCORES = 8
SCALE = HD ** -0.5

NT = N // 128    # 32 j-tiles
NG = NT // 2     # 16 j-tile pairs
NIB = N // 512   # 8 i-blocks
NTB = N // 512   # 8 token blocks (phase 1)
NKT = C // 128   # 6 contraction tiles
NF = 704         # features computed in phase 1 (q01,k01,q22,k22,v012)

# Schraudolph exp in fp16 bits: bits = a*s + b, bitcast int16 -> fp16.
SCH_A = 1024.0 / float(np.log(2.0))
SCH_B = 15321.6
# per-tile engine split: ScalarE cols [0:SPL], VectorE cols [SPL:1024]
SPL = 672


def _build():
    nc = bacc.Bacc("TRN2", target_bir_lowering=False, debug=False,
                   num_devices=NCORES)

    xT = nc.dram_tensor("xT", [C, N], F16, kind="ExternalInput").ap()
    w = nc.dram_tensor("w", [C, NF], F16, kind="ExternalInput").ap()
    bias = nc.dram_tensor("bias", [128, 6], F32, kind="ExternalInput").ap()
    pw = nc.dram_tensor("pw", [HPC * HD, C], F16, kind="ExternalInput").ap()
    bv = nc.dram_tensor("bv", [1, 192], F32, kind="ExternalInput").ap()
    y = nc.dram_tensor("y", [C, N], F16, kind="ExternalOutput").ap()

    xT_r = xT.rearrange("(kt p) (tb n) -> p tb kt n", p=128, n=512)
    w_r = w.rearrange("(kt p) f -> p kt f", p=128)

    with tile.TileContext(nc) as tc:
        with (
            tc.tile_pool(name="singles", bufs=1) as singles,
            tc.tile_pool(name="bigs", bufs=1) as bigs,
            tc.tile_pool(name="xin", bufs=3) as xin,
            tc.tile_pool(name="exp", bufs=6) as expool,
            tc.tile_pool(name="outs", bufs=2) as outs,
            tc.tile_pool(name="psum", bufs=2, space="PSUM") as psum,
            tc.tile_pool(name="dram", bufs=1, space="DRAM") as dram,
        ):
            # ---- constants / weights ----
            w_sb = singles.tile([128, NKT, NF], F16)
            for kt in range(NKT):
                nc.sync.dma_start(out=w_sb[:, kt, :], in_=w_r[:, kt, :])
            bias_sb = singles.tile([128, 6], F32)
            nc.sync.dma_start(out=bias_sb, in_=bias)
            pwa = singles.tile([128, C], F16)
            nc.sync.dma_start(out=pwa, in_=pw[0:128, :])
            pwb = singles.tile([64, C], F16)
            nc.sync.dma_start(out=pwb, in_=pw[128:192, :])
            bv_sb = singles.tile([128, 192], F32)
            nc.sync.dma_start(out=bv_sb, in_=bv.broadcast_to([128, 192]))

            # ---- phase-1 destinations ----
            q01 = bigs.tile([128, N], F16)
            k01 = bigs.tile([128, N], F16)
            q22 = bigs.tile([128, N], F16)
            k22 = bigs.tile([128, N], F16)
            dests = [q01, k01, q22, k22]
            moffs = [0, 128, 256, 384]
            # v per (token-part, head, j-tile): 64 dims + ones col
            vaug = bigs.tile([128, HPC, NT, 65], F16, name="vaug")
            nc.vector.memset(vaug[:, :, :, 64:65], 1.0)

            # ---- phase 1a/1c: q01/k01/q22/k22 (feature-major) ----
            def emit_fgrp(mt, tb, x_t):
                ps = psum.tile([128, 512], F32, tag="sc", bufs=2, name="ps")
                for kt in range(NKT):
                    nc.tensor.matmul(
                        ps,
                        lhsT=w_sb[:, kt, moffs[mt]:moffs[mt] + 128],
                        rhs=x_t[:, kt, :],
                        start=(kt == 0), stop=(kt == NKT - 1),
                    )
                nc.vector.tensor_scalar_add(
                    out=dests[mt][:, tb * 512:(tb + 1) * 512],
                    in0=ps, scalar1=bias_sb[:, mt:mt + 1],
                )

            for tb in range(NTB):
                x_t = xin.tile([128, NKT, 512], F16, bufs=3)
                nc.sync.dma_start(out=x_t, in_=xT_r[:, tb, :, :])
                emit_fgrp(1, tb, x_t)
                emit_fgrp(0, tb, x_t)
                emit_fgrp(3, tb, x_t)
                emit_fgrp(2, tb, x_t)
                for tt in range(4 * tb, 4 * tb + 4):
                    toff = (tt % 4) * 128
                    psv = psum.tile([128, 192], F32, tag="sc", bufs=2)
                    for kt in range(NKT):
                        nc.tensor.matmul(
                            psv,
                            lhsT=x_t[:, kt, toff:toff + 128],
                            rhs=w_sb[:, kt, 512:704],
                            start=(kt == 0), stop=(kt == NKT - 1),
                        )
                    # one add covering all 3 heads: out[h, 0:64] strided
                    nc.vector.tensor_add(
                        vaug[:, :, tt, 0:64],
                        psv,
                        bv_sb,
                    )

            # ---- attention state ----
            o01 = bigs.tile([128, N], F16)
            o2 = bigs.tile([64, N], F16)

            # ---- normalize + proj (interleaved per ib pair) ----
            rb_tiles = {}
            rec_d = dram.tile([48, 512], F32)   # 1/sums bounce for p-bcast

            def emit_rb(ib):
                """broadcast 1/den rows early (DRAM bounce: SBUF DMA reads
                cannot have stride-0 partitions) so they finish before the
                deferred muls consume them."""
                rec_row = 3 * ib
                rb01 = outs.tile([128, 512], F32, tag="rb", bufs=4)
                for h in (0, 1):
                    nc.sync.dma_start(
                        out=rb01[64 * h:64 * h + 64, :],
                        in_=rec_d[rec_row + h:rec_row + h + 1, :]
                        .broadcast_to([64, 512]))
                rb2 = outs.tile([64, 512], F32, tag="rb2", bufs=4)
                nc.sync.dma_start(
                    out=rb2,
                    in_=rec_d[rec_row + 2:rec_row + 3, :]
                    .broadcast_to([64, 512]))
                rb_tiles[ib] = (rb01, rb2)

            def emit_norm(ib, rec_row=None):
                isl = slice(ib * 512, (ib + 1) * 512)
                rb01, rb2 = rb_tiles.pop(ib)
                nc.vector.tensor_mul(o01[:, isl], o01[:, isl], rb01)
                nc.vector.tensor_mul(o2[:, isl], o2[:, isl], rb2)

            def emit_proj(ib, tail=False):
                isl = slice(ib * 512, (ib + 1) * 512)
                for mt in range(6):
                    msl = slice(mt * 128, (mt + 1) * 128)
                    # post-attention the pv banks are dead: alternate psum
                    # tags and split ysb across engines to unserialize
                    tag = "pv0" if (tail and mt % 2 == 1) else "psy"
                    psy = psum.tile([128, 512], F32, tag=tag, bufs=1,
                                    name="psy")
                    nc.tensor.matmul(psy, lhsT=pwa[:, msl], rhs=o01[:, isl],
                                     start=True, stop=False)
                    nc.tensor.matmul(psy, lhsT=pwb[:, msl], rhs=o2[:, isl],
                                     start=False, stop=True)
                    ysb = outs.tile([128, 512], F16, tag="ysb", bufs=4)
                    if tail and mt % 2 == 1:
                        nc.scalar.copy(out=ysb, in_=psy)
                    else:
                        nc.vector.tensor_copy(out=ysb, in_=psy)
                    nc.sync.dma_start(out=y[msl, isl], in_=ysb)


            def emit_exp(sc, out_ap, out_ap_i16):
                """exp(sc[128,1024]) -> fp16 out, split across both engines:
                ScalarE does cols 0:640 (true exp), VectorE does 640:1024
                (Schraudolph bits). Balanced by construction and halves the
                per-tile latency so score psums recycle sooner."""
                nc.scalar.activation(out_ap[:, 0:SPL], sc[:, 0:SPL], EXP)
                nc.vector.tensor_scalar(
                    out=out_ap_i16[:, SPL:1024], in0=sc[:, SPL:1024],
                    scalar1=SCH_A, scalar2=SCH_B,
                    op0=MUL, op1=ADD,
                )

            coll = None
            for ib in range(NIB):
                isl = slice(ib * 512, (ib + 1) * 512)
                pvs = [psum.tile([65, 512], F32, tag=f"pv{h}", bufs=1,
                                 name=f"pv{h}") for h in range(HPC)]
                for g in range(NG):
                    je = slice((2 * g) * 128, (2 * g) * 128 + 128)
                    jo = slice((2 * g + 1) * 128, (2 * g + 1) * 128 + 128)
                    # -- scores: 3 same-tile concurrent pairs, row groups
                    #    alternating L/H so LDWEIGHTS pipelines --
                    scE = psum.tile([128, 1024], F32, tag="sc", bufs=2,
                                    name="scE")
                    scO = psum.tile([128, 1024], F32, tag="sc", bufs=2,
                                    name="scO")
                    scH = psum.tile([128, 1024], F32, tag="sc", bufs=2,
                                    name="scH")
                    nc.tensor.matmul(scE[:, 0:512], lhsT=k01[0:64, je],
                                     rhs=q01[0:64, isl], start=True, stop=True)
                    nc.tensor.matmul(scE[:, 512:1024], lhsT=k01[64:128, je],
                                     rhs=q01[64:128, isl], start=True,
                                     stop=True)
                    nc.tensor.matmul(scO[:, 0:512], lhsT=k01[0:64, jo],
                                     rhs=q01[0:64, isl], start=True, stop=True)
                    nc.tensor.matmul(scO[:, 512:1024], lhsT=k01[64:128, jo],
                                     rhs=q01[64:128, isl], start=True,
                                     stop=True)
                    nc.tensor.matmul(scH[:, 0:512], lhsT=k22[0:64, je],
                                     rhs=q22[0:64, isl], start=True, stop=True)
                    nc.tensor.matmul(scH[:, 512:1024], lhsT=k22[64:128, jo],
                                     rhs=q22[64:128, isl], start=True,
                                     stop=True)

                    # -- exp into exq slots (contiguous [128,1024] each):
                    #    slot0 = jt-even (h0|h1), slot1 = jt-odd (h0|h1),
                    #    slot2 = h2 (je|jo) --
                    exq = expool.tile([128, 3, 1024], F16, name="exq")
                    exq_i16 = exq.bitcast(I16)
                    emit_exp(scE, exq[:, 0, :], exq_i16[:, 0, :])
                    emit_exp(scO, exq[:, 1, :], exq_i16[:, 1, :])
                    emit_exp(scH, exq[:, 2, :], exq_i16[:, 2, :])

                    # -- PV: serial full-K per head --
                    st = (g == 0)
                    sp = (g == NG - 1)
                    for p in range(2):
                        jt = 2 * g + p
                        first = st and p == 0
                        last = sp and p == 1
                        for h in range(HPC):
                            rsl = (slice(h * 512, h * 512 + 512) if h < 2
                                   else slice(p * 512, p * 512 + 512))
                            nc.tensor.matmul(
                                pvs[h], lhsT=vaug[:, h, jt, :],
                                rhs=exq[:, p if h < 2 else 2, rsl],
                                start=first, stop=last)

                # ---- drain: o unnormalized fp16, den rows to coll ----
                if ib % 2 == 0 or ib == NIB - 1:
                    coll = outs.tile([6, 512], F32, tag="coll", bufs=2)
                crow = 0 if ib == NIB - 1 else 3 * (ib % 2)
                nc.vector.tensor_copy(out=o01[0:64, isl], in_=pvs[0][0:64, :])
                nc.vector.tensor_copy(out=o01[64:128, isl], in_=pvs[1][0:64, :])
                nc.scalar.copy(out=o2[:, isl], in_=pvs[2][0:64, :])
                # den staging at partition 64 (engines cross partitions only
                # at 32-aligned offsets), then DMA into coll rows
                for h in range(HPC):
                    sst = outs.tile([65, 512], F32, tag="sst", bufs=3)
                    if h >= 1:
                        nc.scalar.copy(out=sst[64:65, :],
                                       in_=pvs[h][64:65, :])
                    else:
                        nc.vector.tensor_copy(out=sst[64:65, :],
                                              in_=pvs[h][64:65, :])
                    nc.sync.dma_start(out=coll[crow + h:crow + h + 1, :],
                                      in_=sst[64:65, :])

                if ib % 2 == 1 and ib < NIB - 2:
                    rec = outs.tile([6, 512], F32, tag="rec", bufs=3)
                    nc.vector.reciprocal_approx_fast(out=rec, in_=coll)
                    nc.sync.dma_start(
                        out=rec_d[6 * (ib // 2):6 * (ib // 2) + 6, :],
                        in_=rec)
                    emit_rb(ib - 1)
                    emit_rb(ib)
                    # muls+proj for the PREVIOUS pair: their rb broadcasts
                    # finished during this ib's attention, so no DMA stall
                    if ib >= 3:
                        for pb in (ib - 3, ib - 2):
                            emit_norm(pb)
                            emit_proj(pb)
                elif ib == NIB - 2:
                    rec = outs.tile([6, 512], F32, tag="rec", bufs=3)
                    nc.vector.reciprocal_approx_fast(out=rec[0:3, :],
                                                     in_=coll[0:3, :])
                    nc.sync.dma_start(out=rec_d[3 * ib:3 * ib + 3, :],
                                      in_=rec[0:3, :])
                    emit_rb(ib)
                    for pb in (ib - 2, ib - 1):
                        emit_norm(pb)
                        emit_proj(pb)
                elif ib == NIB - 1:
                    rec = outs.tile([6, 512], F32, tag="rec", bufs=3)
                    nc.vector.reciprocal_approx_fast(out=rec[0:3, :],
                                                     in_=coll[0:3, :])
                    nc.sync.dma_start(out=rec_d[3 * ib:3 * ib + 3, :],
                                      in_=rec[0:3, :])
                    emit_rb(ib)
                    emit_norm(ib - 1)
                    emit_proj(ib - 1)
                    emit_norm(ib)
                    emit_proj(ib, tail=True)


    nc.finalize()
    return nc


_NC_CACHE = None


def _get_nc():
    global _NC_CACHE
    if _NC_CACHE is None:
        _NC_CACHE = _build()
    return _NC_CACHE


def _prep_core_inputs(x, qkv_w, qkv_b, proj_w, core):
    """Build the per-core input dict (numpy, host-side)."""
    b, g = core // 4, core % 4
    h = [3 * g, 3 * g + 1, 3 * g + 2]

    xT = np.ascontiguousarray(
        x[b].reshape(N, C).T.astype(np.float16))          # (768, 4096)

    def wq(head):  # scaled q rows, (64, 768)
        return qkv_w[HD * head:HD * (head + 1), :] * SCALE

    def wk(head):
        return qkv_w[C + HD * head:C + HD * (head + 1), :]

    def wv(head):
        return qkv_w[2 * C + HD * head:2 * C + HD * (head + 1), :]

    def bq(head):
        return qkv_b[HD * head:HD * (head + 1)] * SCALE

    def bk(head):
        return qkv_b[C + HD * head:C + HD * (head + 1)]

    def bvv(head):
        return qkv_b[2 * C + HD * head:2 * C + HD * (head + 1)]

    # feature columns: q01 | k01 | q22 | k22 | v012   (704 total)
    wcols = np.concatenate([
        wq(h[0]), wq(h[1]), wk(h[0]), wk(h[1]),
        wq(h[2]), wq(h[2]), wk(h[2]), wk(h[2]),
        wv(h[0]), wv(h[1]), wv(h[2]),
    ], axis=0)                                            # (704, 768)
    w = np.ascontiguousarray(wcols.T.astype(np.float16))  # (768, 704)

    bcols = np.concatenate([
        bq(h[0]), bq(h[1]), bk(h[0]), bk(h[1]),
        bq(h[2]), bq(h[2]), bk(h[2]), bk(h[2]),
        np.zeros(256, np.float32),
    ]).astype(np.float32)                                 # (768,)
    bias = np.ascontiguousarray(bcols.reshape(6, 128).T)  # (128, 6)
    bvec = np.concatenate([bvv(h[0]), bvv(h[1]), bvv(h[2])]
                          ).astype(np.float32).reshape(1, 192)

    ch = slice(HPC * HD * g, HPC * HD * (g + 1))
    pw = np.ascontiguousarray(proj_w[:, ch].T.astype(np.float16))  # (192, 768)

    return {"xT": xT, "w": w, "bias": bias, "pw": pw, "bv": bvec}


def kernel(x, qkv_w, qkv_b, proj_w, proj_b):
    x = np.asarray(x, np.float32)
    qkv_w = np.asarray(qkv_w, np.float32)
    qkv_b = np.asarray(qkv_b, np.float32)
    proj_w = np.asarray(proj_w, np.float32)
    proj_b = np.asarray(proj_b, np.float32)

    nc = _get_nc()
    in_maps = [_prep_core_inputs(x, qkv_w, qkv_b, proj_w, c)
               for c in range(NCORES)]
    res = None
    for attempt in range(3):
        try:
            res = run_bass_kernel_spmd(nc, in_maps, list(range(NCORES)))
            break
        except Exception:
            # transient NEFF-exec failures were observed on first runs;
            # retry before giving up
            if attempt == 2:
                raise

    out = np.empty((B, N, C), np.float32)
    for b in range(B):
        acc = np.zeros((C, N), np.float32)
        for g in range(4):
            acc += res.results[b * 4 + g]["y"].astype(np.float32)
        out[b] = acc.T + proj_b[None, :]
    return out


if __name__ == "__main__":
    rng = np.random.default_rng(0)
    x = rng.standard_normal((B, 64, 64, C), np.float32)
    qkv_w = (rng.standard_normal((3 * C, C), np.float32) * 0.02)
    qkv_b = (rng.standard_normal(3 * C, np.float32) * 0.02)
    proj_w = (rng.standard_normal((C, C), np.float32) * 0.02)
    proj_b = (rng.standard_normal(C, np.float32) * 0.02)
    out = kernel(x=x, qkv_w=qkv_w, qkv_b=qkv_b, proj_w=proj_w, proj_b=proj_b)
    print("out", out.shape, out.dtype, float(np.abs(out).max()))
